# revision 34
# baseline (speedup 1.0000x reference)
"""Trainium2 Bass kernel for 2D attention with relative-position augmentation.

Problem shapes (hardcoded): inputs [8, 32, 32, 768] fp32 (q|k|v packed on the
channel axis, 256 each), key_rel_w/key_rel_h [63, 32] fp32.
Output: [8, 32, 32, 256] fp32.

Sharding: data-parallel over batch - core b gets batch b (8 cores, no
collectives needed).

Per-core math (N = 32*32 = 1024 tokens, 8 heads, head dim 32):
  L[n, m] = Q[n].K[m] + qdw[n, y2(m)-y(n)+31] + qdh[n, x2(m)-x(n)+31]
  out[n]  = softmax_m(L[n, :] / sqrt(32)) @ V
where qdw = Q @ key_rel_w^T, qdh = Q @ key_rel_h^T and n=(x,y), m=(x2,y2).

Kernel formulation (v2):
  * L^T is computed (m on partitions, n free) with the two relative-logit
    terms folded into the same matmul by extending the contraction dim from
    32 to 96:
       lhsT rows  0-31: K^T            rhs rows  0-31: Q^T
       lhsT rows 32-63: Aw[y',m]=[y2(m)==y']   rhs rows 32-63: Bw[y',n]=qdw^T[y'-y(n)+31, n]
       lhsT rows 64-95: Ah[x',m]=[x2(m)==x']   rhs rows 64-95: Bh[x',n]=qdh^T[x'-x(n)+31, n]
  * P^T = exp(L^T / sqrt(32)) (scale folded into the Exp activation); no
    max-subtraction (logits are small).
  * AV uses V as the STATIONARY operand (33-column weight loads) and P^T as
    the moving operand, producing out^T[c, n] in PSUM accumulated over the 8
    m-chunks.  A ones column appended to V yields the softmax denominator
    s[n] as psum row 32.
  * Normalization: DVE reciprocal of row 32 -> gpsimd partition_broadcast ->
    DVE multiply.  Output is written c-major [256, 1024] and transposed on
    the host during the gather step.
  * The main loop is software-pipelined 2 deep (PE stream: L0 L1 AV0 L2
    AV1 ...) so the Exp latency hides behind PE work with only 2 L-psum
    buffers (PSUM: 2x l[128,1024] + 2x o[33,1024] = 8 banks).
"""

import numpy as np

import concourse.bacc as bacc
import concourse.mybir as mybir
from concourse.tile import TileContext
from concourse.bass_utils import run_bass_kernel_spmd



F32 = mybir.dt.float32
BF16 = mybir.dt.bfloat16
I32 = mybir.dt.int32
AF = mybir.ActivationFunctionType
ALU = mybir.AluOpType

N_CORES = 8
N = 1024          # tokens per batch (32 x 32)
NH = 8            # heads
EXP_SCALE = float(1.0 / np.sqrt(32.0))
# Schraudolph-style exp for the DVE-offloaded chunks:
#   exp(x/sqrt(32)) ~ bitcast_f32(int32(A*x + B)); C tuned for zero mean error
SCHRAUD_A = float((1 << 23) / np.log(2.0) / np.sqrt(32.0))
SCHRAUD_B = float((127 << 23) - 480000)
DVE_EXP_I = ()  # chunks per head computed on DVE instead of ACT

_CACHE = {}


def _emit(tc, x, rw, rh, out):
    nc = tc.nc

    with tc.tile_pool(name="big", bufs=1) as big, \
         tc.tile_pool(name="dram", bufs=1, space="DRAM") as dram:

        # ---- identity for PE transposes (gpsimd iota early: Q path gate)
        iti = big.tile([128, 128], I32, name="iti")
        nc.gpsimd.iota(iti[:], pattern=[[1, 128]], base=0, channel_multiplier=-1)
        identf = big.tile([128, 128], F32, name="identf")
        nc.vector.tensor_scalar(identf[:], iti[:], 0, None, ALU.is_equal)

        # ---- Q/K/V natural fp32 loads (q split in col halves so the PE
        # transposes of block 0 can start as soon as its half lands); Q and K
        # are PE-transposed on chip -- no DRAM cast round-trip, and few DMAs
        # (a large DMA count causes false serialization through the reused
        # completion-semaphore pool).
        x_r = x.rearrange("(t p) c -> p t c", p=128)
        xq = big.tile([128, 8 * 256], F32, name="xq")
        xq_r = xq[:].rearrange("p (t c) -> p t c", c=256)
        for cb in range(2):
            nc.sync.dma_start(
                out=xq_r[:, :, cb * 128:(cb + 1) * 128],
                in_=x_r[:, :, cb * 128:(cb + 1) * 128])
        xk = big.tile([128, 8 * 256], F32, name="xk")
        xk_r = xk[:].rearrange("p (t c) -> p t c", c=256)
        for cb in range(2):
            nc.scalar.dma_start(
                out=xk_r[:, :, cb * 128:(cb + 1) * 128],
                in_=x_r[:, :, 256 + cb * 128:256 + (cb + 1) * 128])
        xv = big.tile([128, 8 * 256], F32, name="xv")
        nc.scalar.dma_start(
            out=xv[:].rearrange("p (t c) -> p t c", c=256), in_=x_r[:, :, 512:768])
        vp = big.tile([128, 8 * NH * 33], BF16, name="vp")
        nc.gpsimd.dma_start(
            out=vp[:].rearrange("p (g c) -> p g c", c=33)[:, :, 0:32],
            in_=xv[:].rearrange("p (g c) -> p g c", c=32),
        )

        # ---- rel tables -> RT [32, 128] bf16 (cols 0-62 w-table^T, 64-126
        # h-table^T; cols 63/127 zero)
        rel4 = big.tile([32, 128], F32, name="rel4")
        nc.vector.memset(rel4[:, :], 0.0)
        nc.scalar.dma_start(out=rel4[0:32, 0:32], in_=rw[0:32, :])
        nc.scalar.dma_start(out=rel4[0:31, 32:64], in_=rw[32:63, :])
        nc.scalar.dma_start(out=rel4[0:32, 64:96], in_=rh[0:32, :])
        nc.scalar.dma_start(out=rel4[0:31, 96:128], in_=rh[32:63, :])
        rtf = big.tile([32, 128], F32, name="rtf")
        nc.vector.transpose(rtf[:, :], rel4[:, :])  # 4x 32x32 block transpose
        rt = big.tile([32, 128], BF16, name="rt")
        nc.vector.tensor_copy(rt[:], rtf[:])

        # ---- extended operand tiles.  ke rows 32-95 are the one-hot
        # selectors (same for every head): write head-0 block then replicate.
        ke = big.tile([96, NH * N], BF16, name="ke")
        qe = big.tile([96, NH * N], BF16, name="qe")
        itw = big.tile([32, N], I32, name="itw")
        ith = big.tile([32, N], I32, name="ith")
        nc.gpsimd.iota(
            itw[:].rearrange("p (mx my) -> p mx my", mx=32),
            pattern=[[0, 32], [1, 32]], base=0, channel_multiplier=-1,
        )
        nc.gpsimd.iota(
            ith[:].rearrange("p (mx my) -> p mx my", mx=32),
            pattern=[[1, 32], [0, 32]], base=0, channel_multiplier=-1,
        )
        nc.vector.tensor_scalar(ke[32:64, 0:N], itw[:], 0, None, ALU.is_equal)
        nc.vector.tensor_scalar(ke[64:96, 0:N], ith[:], 0, None, ALU.is_equal)


        qe_v = qe[:].rearrange("p (h nx ny) -> p h nx ny", h=NH, nx=32)

        with tc.tile_pool(name="ptp", bufs=4) as ptp, \
             tc.tile_pool(name="nr1", bufs=2) as nr1, \
             tc.tile_pool(name="nr2", bufs=2) as nr2, \
             tc.tile_pool(name="outp", bufs=2) as outp:

            # ---- Q^T/K^T via PE transposes of the natural fp32 loads,
            # scattered per-head into qe/ke by DVE/ACT cast-copies.
            with tc.tile_pool(name="trp", bufs=2, space="PSUM") as trp:
                for src_t, dst_t in ((xq_r, qe), (xk_r, ke)):
                    for cb in range(2):
                        tt = trp.tile([128, N], F32, name="tr")
                        for t in range(8):
                            nc.tensor.transpose(
                                tt[:, t * 128:(t + 1) * 128],
                                src_t[:, t, cb * 128:(cb + 1) * 128],
                                identf[:],
                            )
                        for hh in range(4):
                            h = cb * 4 + hh
                            dst = dst_t[0:32, h * N:(h + 1) * N]
                            src = tt[hh * 32:(hh + 1) * 32, :]
                            if hh % 2 == 0:
                                nc.vector.tensor_copy(dst, src)
                            else:
                                nc.scalar.copy(dst, src)
            for r in range(NH - 1):
                nc.sync.dma_start(
                    out=ke[32:96, (r + 1) * N:(r + 2) * N], in_=ke[32:96, 0:N]
                )
            vp_r = vp[:].rearrange("p (t h c) -> p t h c", t=8, h=NH)
            nc.vector.memset(vp_r[:, :, :, 32:33], 1.0)

            with tc.tile_pool(name="pp", bufs=3, space="PSUM") as pp, \
                 tc.tile_pool(name="op", bufs=1, space="PSUM") as op:
                _emit_main(tc, pp, op, ptp, nr1, nr2, outp, ke, qe, qe_v, rt, vp, out)


def _emit_main(tc, pp, op, ptp, nr1, nr2, outp, ke, qe, qe_v, rt, vp, out):
    nc = tc.nc
    if True:
        if True:
            # ---- B phase: Bw/Bh rows of qe, all 8 heads per matmul.
            #   Bw[y', n]|y(n)=v = rel_w[31-v+y'] . Q[n] -> lhsT = rt[:, 31-v:63-v]
            #   Bh[x', n]|x(n)=v = rel_h[31-v+x'] . Q[n] -> lhsT = rt[:, 95-v:127-v]
            for g in range(8):
                b_ps = pp.tile([128, 1024], F32, name="l_ps")
                for dy in range(4):
                    v = 4 * g + dy
                    nc.tensor.matmul(
                        b_ps[0:32, dy * 256:(dy + 1) * 256],
                        rt[:, 31 - v:63 - v],
                        qe_v[0:32, :, :, v:v + 1],
                        start=True, stop=True,
                    )
                    nc.tensor.matmul(
                        b_ps[32:64, dy * 256:(dy + 1) * 256],
                        rt[:, 95 - v:127 - v],
                        qe_v[0:32, :, v:v + 1, :],
                        start=True, stop=True,
                    )
                bw = b_ps[0:32, :].rearrange("p (y h x) -> p h x y", y=4, h=NH)
                bh = b_ps[32:64, :].rearrange("p (x h y) -> p h x y", x=4, h=NH)
                if g % 2 == 0:
                    nc.vector.tensor_copy(qe_v[32:64, :, :, 4 * g:4 * g + 4], bw)
                    nc.scalar.copy(qe_v[64:96, :, 4 * g:4 * g + 4, :], bh)
                else:
                    nc.scalar.copy(qe_v[32:64, :, :, 4 * g:4 * g + 4], bw)
                    nc.vector.tensor_copy(qe_v[64:96, :, 4 * g:4 * g + 4, :], bh)

            # ---- main loop, software-pipelined 2 deep over 64 (head, chunk)
            # pairs: PE stream is L0 L1 AV0 L2 AV1 ... L63 AV62 AV63.
            chunks = [(h, i) for h in range(NH) for i in range(8)]
            l_tiles = [None] * 64
            pt_tiles = [None] * 64

            def emit_l(j):
                h, i = chunks[j]
                lp = pp.tile([128, 1024], F32, name="l_ps")
                for c in range(2):
                    nc.tensor.matmul(
                        lp[:, c * 512:(c + 1) * 512],
                        ke[:, h * N + i * 128: h * N + i * 128 + 128],
                        qe[:, h * N + c * 512: h * N + (c + 1) * 512],
                        start=True, stop=True,
                    )
                l_tiles[j] = lp

            emit_l(0)
            emit_l(1)
            emit_l(2)
            o_ps = None
            for j, (h, i) in enumerate(chunks):
                pt = ptp.tile([128, N], BF16, name="pt")
                if i in DVE_EXP_I:
                    it = ptp.tile([128, N], I32, name="it")
                    nc.vector.tensor_scalar(
                        it[:], l_tiles[j][:], SCHRAUD_A, SCHRAUD_B,
                        ALU.mult, ALU.add,
                    )
                    nc.vector.tensor_copy(pt[:], it[:].bitcast(F32))
                else:
                    nc.scalar.activation(
                        pt[:], l_tiles[j][:], AF.Exp, scale=EXP_SCALE)
                pt_tiles[j] = pt
                l_tiles[j] = None
                if i == 0:
                    o_ps = op.tile([33, 1024], F32, name="o_ps")
                for c in range(2):
                    nc.tensor.matmul(
                        o_ps[:, c * 512:(c + 1) * 512],
                        vp[:, (i * NH + h) * 33:(i * NH + h) * 33 + 33],
                        pt[:, c * 512:(c + 1) * 512],
                        start=(i == 0), stop=(i == 7),
                    )
                if j + 3 < 64:
                    emit_l(j + 3)
                if i == 7:
                    # pull [A; s] out of PSUM immediately (frees the single
                    # o_ps slot), then normalize on SBUF tiles:
                    # out[c, n] = A[c, n] / s[n]
                    asb = outp.tile([33, 1024], F32, name="asb")
                    nc.vector.tensor_copy(asb[:], o_ps[:])
                    ssr = nr1.tile([1, 1024], F32, name="ssr")
                    nc.vector.tensor_copy(ssr[:], asb[32:33, :])
                    rs = nr1.tile([1, 1024], F32, name="rs")
                    nc.vector.reciprocal_approx_fast(rs[:], ssr[:])
                    rb = nr2.tile([32, 1024], F32, name="rb")
                    nc.gpsimd.partition_broadcast(rb[:], rs[:])
                    ot = outp.tile([32, 1024], F32, name="ot")
                    nc.vector.tensor_mul(ot[:], asb[0:32, :], rb[:])
                    nc.sync.dma_start(
                        out=out[h * 32:(h + 1) * 32, :], in_=ot[:]
                    )


def build_nc():
    if "nc" in _CACHE:
        return _CACHE["nc"]
    nc = bacc.Bacc(
        "TRN2", target_bir_lowering=False, debug=False, num_devices=N_CORES
    )
    x = nc.dram_tensor("x", [N, 768], F32, kind="ExternalInput")
    rw = nc.dram_tensor("rw", [63, 32], F32, kind="ExternalInput")
    rh = nc.dram_tensor("rh", [63, 32], F32, kind="ExternalInput")
    out = nc.dram_tensor("out", [256, N], F32, kind="ExternalOutput")
    with TileContext(nc) as tc:
        _emit(tc, x.ap(), rw.ap(), rh.ap(), out.ap())
    nc.compile()
    _CACHE["nc"] = nc
    return nc


def kernel(inputs, key_rel_w, key_rel_h):
    B = inputs.shape[0]
    assert inputs.shape == (8, 32, 32, 768), inputs.shape
    nc = build_nc()
    x_full = np.ascontiguousarray(inputs.reshape(B, N, 768), dtype=np.float32)
    rw = np.ascontiguousarray(key_rel_w, dtype=np.float32)
    rh = np.ascontiguousarray(key_rel_h, dtype=np.float32)
    in_maps = [{"x": x_full[b], "rw": rw, "rh": rh} for b in range(N_CORES)]
    res = run_bass_kernel_spmd(nc, in_maps, list(range(N_CORES)))
    return np.stack(
        [res.results[b]["out"].T.reshape(32, 32, 256) for b in range(N_CORES)]
    )


if __name__ == "__main__":
    rng = np.random.default_rng(0)
    inputs = rng.standard_normal((8, 32, 32, 768), dtype=np.float32)
    rw = rng.standard_normal((63, 32), dtype=np.float32) * 32 ** -0.5
    rh = rng.standard_normal((63, 32), dtype=np.float32) * 32 ** -0.5
    o = kernel(inputs, rw, rh)
    print(o.shape, o.dtype, float(np.abs(o).max()))


# revision 35
# speedup vs baseline: 1.0202x; 1.0202x over previous
"""Trainium2 Bass kernel for 2D attention with relative-position augmentation.

Problem shapes (hardcoded): inputs [8, 32, 32, 768] fp32 (q|k|v packed on the
channel axis, 256 each), key_rel_w/key_rel_h [63, 32] fp32.
Output: [8, 32, 32, 256] fp32.

Sharding: data-parallel over batch - core b gets batch b (8 cores, no
collectives needed).

Per-core math (N = 32*32 = 1024 tokens, 8 heads, head dim 32):
  L[n, m] = Q[n].K[m] + qdw[n, y2(m)-y(n)+31] + qdh[n, x2(m)-x(n)+31]
  out[n]  = softmax_m(L[n, :] / sqrt(32)) @ V
where qdw = Q @ key_rel_w^T, qdh = Q @ key_rel_h^T and n=(x,y), m=(x2,y2).

Kernel formulation (v2):
  * L^T is computed (m on partitions, n free) with the two relative-logit
    terms folded into the same matmul by extending the contraction dim from
    32 to 96:
       lhsT rows  0-31: K^T            rhs rows  0-31: Q^T
       lhsT rows 32-63: Aw[y',m]=[y2(m)==y']   rhs rows 32-63: Bw[y',n]=qdw^T[y'-y(n)+31, n]
       lhsT rows 64-95: Ah[x',m]=[x2(m)==x']   rhs rows 64-95: Bh[x',n]=qdh^T[x'-x(n)+31, n]
  * P^T = exp(L^T / sqrt(32)) (scale folded into the Exp activation); no
    max-subtraction (logits are small).
  * AV uses V as the STATIONARY operand (33-column weight loads) and P^T as
    the moving operand, producing out^T[c, n] in PSUM accumulated over the 8
    m-chunks.  A ones column appended to V yields the softmax denominator
    s[n] as psum row 32.
  * Normalization: DVE reciprocal of row 32 -> gpsimd partition_broadcast ->
    DVE multiply.  Output is written c-major [256, 1024] and transposed on
    the host during the gather step.
  * The main loop is software-pipelined 2 deep (PE stream: L0 L1 AV0 L2
    AV1 ...) so the Exp latency hides behind PE work with only 2 L-psum
    buffers (PSUM: 2x l[128,1024] + 2x o[33,1024] = 8 banks).
"""

import numpy as np

import concourse.bacc as bacc
import concourse.mybir as mybir
from concourse.tile import TileContext
from concourse.bass_utils import run_bass_kernel_spmd



F32 = mybir.dt.float32
BF16 = mybir.dt.bfloat16
I32 = mybir.dt.int32
AF = mybir.ActivationFunctionType
ALU = mybir.AluOpType

N_CORES = 8
N = 1024          # tokens per batch (32 x 32)
NH = 8            # heads
EXP_SCALE = float(1.0 / np.sqrt(32.0))
# Schraudolph-style exp for the DVE-offloaded chunks:
#   exp(x/sqrt(32)) ~ bitcast_f32(int32(A*x + B)); C tuned for zero mean error
SCHRAUD_A = float((1 << 23) / np.log(2.0) / np.sqrt(32.0))
SCHRAUD_B = float((127 << 23) - 480000)
DVE_EXP_I = ()  # chunks per head computed on DVE instead of ACT

_CACHE = {}


def _emit(tc, x, rw, rh, out):
    nc = tc.nc

    with tc.tile_pool(name="big", bufs=1) as big, \
         tc.tile_pool(name="dram", bufs=1, space="DRAM") as dram:

        # ---- identity for PE transposes (gpsimd iota early: Q path gate)
        iti = big.tile([128, 128], I32, name="iti")
        nc.gpsimd.iota(iti[:], pattern=[[1, 128]], base=0, channel_multiplier=-1)
        identf = big.tile([128, 128], F32, name="identf")
        nc.vector.tensor_scalar(identf[:], iti[:], 0, None, ALU.is_equal)

        # ---- Q/K/V natural fp32 loads (q split in col halves so the PE
        # transposes of block 0 can start as soon as its half lands); Q and K
        # are PE-transposed on chip -- no DRAM cast round-trip, and few DMAs
        # (a large DMA count causes false serialization through the reused
        # completion-semaphore pool).
        x_r = x.rearrange("(t p) c -> p t c", p=128)
        xq = big.tile([128, 8 * 256], F32, name="xq")
        xq_r = xq[:].rearrange("p (t c) -> p t c", c=256)
        for cb in range(2):
            nc.sync.dma_start(
                out=xq_r[:, :, cb * 128:(cb + 1) * 128],
                in_=x_r[:, :, cb * 128:(cb + 1) * 128])
        xk = big.tile([128, 8 * 256], F32, name="xk")
        xk_r = xk[:].rearrange("p (t c) -> p t c", c=256)
        for cb in range(2):
            nc.scalar.dma_start(
                out=xk_r[:, :, cb * 128:(cb + 1) * 128],
                in_=x_r[:, :, 256 + cb * 128:256 + (cb + 1) * 128])
        xv = big.tile([128, 8 * 256], F32, name="xv")
        nc.scalar.dma_start(
            out=xv[:].rearrange("p (t c) -> p t c", c=256), in_=x_r[:, :, 512:768])
        vp = big.tile([128, 8 * NH * 33], BF16, name="vp")

        # ---- rel tables -> RT [32, 128] bf16 (cols 0-62 w-table^T, 64-126
        # h-table^T; cols 63/127 zero)
        rel4 = big.tile([32, 128], F32, name="rel4")
        nc.vector.memset(rel4[:, :], 0.0)
        nc.scalar.dma_start(out=rel4[0:32, 0:32], in_=rw[0:32, :])
        nc.scalar.dma_start(out=rel4[0:31, 32:64], in_=rw[32:63, :])
        nc.scalar.dma_start(out=rel4[0:32, 64:96], in_=rh[0:32, :])
        nc.scalar.dma_start(out=rel4[0:31, 96:128], in_=rh[32:63, :])
        rtf = big.tile([32, 128], F32, name="rtf")
        nc.vector.transpose(rtf[:, :], rel4[:, :])  # 4x 32x32 block transpose
        rt = big.tile([32, 128], BF16, name="rt")
        nc.vector.tensor_copy(rt[:], rtf[:])

        # ---- extended operand tiles.  ke rows 32-95 are the one-hot
        # selectors (same for every head): write head-0 block then replicate.
        ke = big.tile([96, NH * N], BF16, name="ke")
        qe = big.tile([96, NH * N], BF16, name="qe")
        itw = big.tile([32, N], I32, name="itw")
        ith = big.tile([32, N], I32, name="ith")
        nc.gpsimd.iota(
            itw[:].rearrange("p (mx my) -> p mx my", mx=32),
            pattern=[[0, 32], [1, 32]], base=0, channel_multiplier=-1,
        )
        nc.gpsimd.iota(
            ith[:].rearrange("p (mx my) -> p mx my", mx=32),
            pattern=[[1, 32], [0, 32]], base=0, channel_multiplier=-1,
        )
        nc.vector.tensor_scalar(ke[32:64, 0:N], itw[:], 0, None, ALU.is_equal)
        nc.vector.tensor_scalar(ke[64:96, 0:N], ith[:], 0, None, ALU.is_equal)


        qe_v = qe[:].rearrange("p (h nx ny) -> p h nx ny", h=NH, nx=32)

        with tc.tile_pool(name="ptp", bufs=4) as ptp, \
             tc.tile_pool(name="nr1", bufs=2) as nr1, \
             tc.tile_pool(name="nr2", bufs=2) as nr2, \
             tc.tile_pool(name="outp", bufs=2) as outp:

            # ---- Q^T/K^T via PE transposes of the natural fp32 loads,
            # scattered per-head into qe/ke by DVE/ACT cast-copies.
            with tc.tile_pool(name="trp", bufs=2, space="PSUM") as trp:
                for src_t, dst_t in ((xq_r, qe), (xk_r, ke)):
                    for cb in range(2):
                        tt = trp.tile([128, N], F32, name="tr")
                        for t in range(8):
                            nc.tensor.transpose(
                                tt[:, t * 128:(t + 1) * 128],
                                src_t[:, t, cb * 128:(cb + 1) * 128],
                                identf[:],
                            )
                        for hh in range(4):
                            h = cb * 4 + hh
                            dst = dst_t[0:32, h * N:(h + 1) * N]
                            src = tt[hh * 32:(hh + 1) * 32, :]
                            if hh % 2 == 0:
                                nc.vector.tensor_copy(dst, src)
                            else:
                                nc.scalar.copy(dst, src)
            for r in range(NH - 1):
                nc.sync.dma_start(
                    out=ke[32:96, (r + 1) * N:(r + 2) * N], in_=ke[32:96, 0:N]
                )
            vp_r = vp[:].rearrange("p (t h c) -> p t h c", t=8, h=NH)
            xv_r = xv[:].rearrange("p (t h c) -> p t h c", t=8, h=NH)
            nc.vector.tensor_copy(vp_r[:, :, :, 0:32], xv_r)
            nc.vector.memset(vp_r[:, :, :, 32:33], 1.0)

            with tc.tile_pool(name="pp", bufs=3, space="PSUM") as pp, \
                 tc.tile_pool(name="op", bufs=1, space="PSUM") as op:
                _emit_main(tc, pp, op, ptp, nr1, nr2, outp, ke, qe, qe_v, rt, vp, out)


def _emit_main(tc, pp, op, ptp, nr1, nr2, outp, ke, qe, qe_v, rt, vp, out):
    nc = tc.nc
    if True:
        if True:
            # ---- B phase: Bw/Bh rows of qe, all 8 heads per matmul.
            #   Bw[y', n]|y(n)=v = rel_w[31-v+y'] . Q[n] -> lhsT = rt[:, 31-v:63-v]
            #   Bh[x', n]|x(n)=v = rel_h[31-v+x'] . Q[n] -> lhsT = rt[:, 95-v:127-v]
            for g in range(8):
                b_ps = pp.tile([128, 1024], F32, name="l_ps")
                for dy in range(4):
                    v = 4 * g + dy
                    nc.tensor.matmul(
                        b_ps[0:32, dy * 256:(dy + 1) * 256],
                        rt[:, 31 - v:63 - v],
                        qe_v[0:32, :, :, v:v + 1],
                        start=True, stop=True,
                    )
                    nc.tensor.matmul(
                        b_ps[32:64, dy * 256:(dy + 1) * 256],
                        rt[:, 95 - v:127 - v],
                        qe_v[0:32, :, v:v + 1, :],
                        start=True, stop=True,
                    )
                bw = b_ps[0:32, :].rearrange("p (y h x) -> p h x y", y=4, h=NH)
                bh = b_ps[32:64, :].rearrange("p (x h y) -> p h x y", x=4, h=NH)
                if g % 2 == 0:
                    nc.vector.tensor_copy(qe_v[32:64, :, :, 4 * g:4 * g + 4], bw)
                    nc.scalar.copy(qe_v[64:96, :, 4 * g:4 * g + 4, :], bh)
                else:
                    nc.scalar.copy(qe_v[32:64, :, :, 4 * g:4 * g + 4], bw)
                    nc.vector.tensor_copy(qe_v[64:96, :, 4 * g:4 * g + 4, :], bh)

            # ---- main loop, software-pipelined 2 deep over 64 (head, chunk)
            # pairs: PE stream is L0 L1 AV0 L2 AV1 ... L63 AV62 AV63.
            chunks = [(h, i) for h in range(NH) for i in range(8)]
            l_tiles = [None] * 64
            pt_tiles = [None] * 64

            def emit_l(j):
                h, i = chunks[j]
                lp = pp.tile([128, 1024], F32, name="l_ps")
                for c in range(2):
                    nc.tensor.matmul(
                        lp[:, c * 512:(c + 1) * 512],
                        ke[:, h * N + i * 128: h * N + i * 128 + 128],
                        qe[:, h * N + c * 512: h * N + (c + 1) * 512],
                        start=True, stop=True,
                    )
                l_tiles[j] = lp

            emit_l(0)
            emit_l(1)
            emit_l(2)
            o_ps = None
            for j, (h, i) in enumerate(chunks):
                pt = ptp.tile([128, N], BF16, name="pt")
                if i in DVE_EXP_I:
                    it = ptp.tile([128, N], I32, name="it")
                    nc.vector.tensor_scalar(
                        it[:], l_tiles[j][:], SCHRAUD_A, SCHRAUD_B,
                        ALU.mult, ALU.add,
                    )
                    nc.vector.tensor_copy(pt[:], it[:].bitcast(F32))
                else:
                    nc.scalar.activation(
                        pt[:], l_tiles[j][:], AF.Exp, scale=EXP_SCALE)
                pt_tiles[j] = pt
                l_tiles[j] = None
                if i == 0:
                    o_ps = op.tile([33, 1024], F32, name="o_ps")
                for c in range(2):
                    nc.tensor.matmul(
                        o_ps[:, c * 512:(c + 1) * 512],
                        vp[:, (i * NH + h) * 33:(i * NH + h) * 33 + 33],
                        pt[:, c * 512:(c + 1) * 512],
                        start=(i == 0), stop=(i == 7),
                    )
                if j + 3 < 64:
                    emit_l(j + 3)
                if i == 7:
                    # pull [A; s] out of PSUM immediately (frees the single
                    # o_ps slot), then normalize on SBUF tiles:
                    # out[c, n] = A[c, n] / s[n]
                    asb = outp.tile([33, 1024], F32, name="asb")
                    nc.vector.tensor_copy(asb[:], o_ps[:])
                    ssr = nr1.tile([1, 1024], F32, name="ssr")
                    nc.vector.tensor_copy(ssr[:], asb[32:33, :])
                    rs = nr1.tile([1, 1024], F32, name="rs")
                    nc.vector.reciprocal_approx_fast(rs[:], ssr[:])
                    rb = nr2.tile([32, 1024], F32, name="rb")
                    nc.gpsimd.partition_broadcast(rb[:], rs[:])
                    ot = outp.tile([32, 1024], F32, name="ot")
                    nc.vector.tensor_mul(ot[:], asb[0:32, :], rb[:])
                    nc.sync.dma_start(
                        out=out[h * 32:(h + 1) * 32, :], in_=ot[:]
                    )


def build_nc():
    if "nc" in _CACHE:
        return _CACHE["nc"]
    nc = bacc.Bacc(
        "TRN2", target_bir_lowering=False, debug=False, num_devices=N_CORES
    )
    x = nc.dram_tensor("x", [N, 768], F32, kind="ExternalInput")
    rw = nc.dram_tensor("rw", [63, 32], F32, kind="ExternalInput")
    rh = nc.dram_tensor("rh", [63, 32], F32, kind="ExternalInput")
    out = nc.dram_tensor("out", [256, N], F32, kind="ExternalOutput")
    with TileContext(nc) as tc:
        _emit(tc, x.ap(), rw.ap(), rh.ap(), out.ap())
    nc.compile()
    _CACHE["nc"] = nc
    return nc


def kernel(inputs, key_rel_w, key_rel_h):
    B = inputs.shape[0]
    assert inputs.shape == (8, 32, 32, 768), inputs.shape
    nc = build_nc()
    x_full = np.ascontiguousarray(inputs.reshape(B, N, 768), dtype=np.float32)
    rw = np.ascontiguousarray(key_rel_w, dtype=np.float32)
    rh = np.ascontiguousarray(key_rel_h, dtype=np.float32)
    in_maps = [{"x": x_full[b], "rw": rw, "rh": rh} for b in range(N_CORES)]
    res = run_bass_kernel_spmd(nc, in_maps, list(range(N_CORES)))
    return np.stack(
        [res.results[b]["out"].T.reshape(32, 32, 256) for b in range(N_CORES)]
    )


if __name__ == "__main__":
    rng = np.random.default_rng(0)
    inputs = rng.standard_normal((8, 32, 32, 768), dtype=np.float32)
    rw = rng.standard_normal((63, 32), dtype=np.float32) * 32 ** -0.5
    rh = rng.standard_normal((63, 32), dtype=np.float32) * 32 ** -0.5
    o = kernel(inputs, rw, rh)
    print(o.shape, o.dtype, float(np.abs(o).max()))
